# revision 8
# baseline (speedup 1.0000x reference)
"""Multi-head distance (attention) layer on 8 TRN2 NeuronCores.

Sharding: data-parallel over batch. B=8 -> one batch element per core.
Each core computes a full multi-head self-attention for its [L=1024, D=256]
slice with H=8 heads of dim 64. No collectives needed.

Per-core algorithm (fp16 matmul operands; ScalarE exp is the pacing engine):
  xT    = DMA-xbar transpose of x (HBM -> SBUF, no PE involvement)
  qkT   = xT + peT               (pos-enc, host-precomputed, DVE)
  qT    = Wq.T @ qkT + bq        per head-pair tiles [128(d), 1024(l)]
  kT2   = Wk.T @ qkT             per head-pair tiles [128(d), 1024(m)]
  v     = xT.T @ Wv              [m, 8, 64+1] tiles, ones column for Z
  S^T   = per-head kT2/qT matmuls, K=64, issued as CONCURRENT row-tiled
          pairs (heads 2j/2j+1 on PE row-groups 0-1/2-3 via base_partition
          0/64) -> 2x S throughput vs the zero-padded K=128 formulation.
  exp   = ScalarE, FD=1536 reads from a 6-bank PSUM ring ([128, 3072] f32)
          that all matmul chunks ([128, 512] = 1 bank) allocate through
          sequentially. Projection chunks interleave in mod-3 blocks (with
          ring-position skips) so every exp read is contiguous at ring
          offset 0 or 1536. 43 exp calls instead of 64 -> ~6.5us less
          ScalarE busy; ScalarE does nothing else.
  O     = e.T @ [v_h | 1] accumulated over 8 m-chunks into [128, 260] PSUM
          (2 banks double-buffered), normalized by DVE reciprocal+mul,
          DMA'd out per (head, quad). The final head-pair's q=1 quads are
          emitted eagerly per-exp so the tail after the last exp is short.
Bias handling: bq added during the Q PSUM drain (DVE, per-partition
scalar); bk only shifts score rows by a constant (softmax-invariant) so it
is dropped; bv shifts the output by exactly repeat(bv, 64) because softmax
rows sum to 1, added on the host.
"""

import numpy as np
import ml_dtypes

import concourse.bass as bass
import concourse.mybir as mybir
import concourse.tile as tile
from concourse import bacc
from concourse.bass_utils import run_bass_kernel_spmd

B, L, D = 8, 1024, 256
H, HD = 8, 64
J = H * HD  # 512
TEMPERATURE = 10000.0

f32 = mybir.dt.float32
bf16 = mybir.dt.float16  # fp16: same PE rate as bf16, 8x the mantissa

_CACHE = {}
LAST_RESULT = None  # BassKernelResults of the most recent run (for profiling)
TRACE = False

RING_CHUNKS = 6   # 6 PSUM banks of [128, 512] f32
EXP_CHUNKS = 3    # FD=1536 per ScalarE exp call


def _emit(tc, aps):
    nc = tc.nc
    Exp = mybir.ActivationFunctionType.Exp
    x, wq, wk, wv, bqc, pet, out = (
        aps["x"], aps["wq"], aps["wk"], aps["wv"], aps["bqc"], aps["pet"], aps["out"],
    )

    petr = pet.rearrange("(t p) l -> t p l", p=128)      # [2, 128, 1024]
    wqr = wq.rearrange("(t p) j -> t p j", p=128)        # [2, 128, 512]
    wkr = wk.rearrange("(t p) j -> t p j", p=128)
    wvr = wv.rearrange("(t p) j -> t p j", p=128)
    outr = out.rearrange("(n p) j -> p n j", p=128)      # [128, 8, 512]

    import contextlib
    ctx = contextlib.ExitStack()
    persist = ctx.enter_context(tc.tile_pool(name="persist", bufs=1))
    epool = ctx.enter_context(tc.tile_pool(name="epool", bufs=10))
    rpool = ctx.enter_context(tc.tile_pool(name="rpool", bufs=4))
    ring_pool = ctx.enter_context(tc.tile_pool(name="ringp", bufs=1, space="PSUM"))
    o_ps = ctx.enter_context(tc.tile_pool(name="ops", bufs=2, space="PSUM"))

    # --- ACT exp-table preload (off the attention critical path) ---
    sc_in = persist.tile([128, 8], f32, name="sc_in")
    sc_out = persist.tile([128, 8], f32, name="sc_out")
    nc.vector.memset(sc_in[:], 0.0)
    nc.scalar.activation(sc_out[:], sc_in[:], Exp)

    # --- input DMAs. xT via the DMA xbar transpose (no PE); sync + scalar
    # host the two HWDGE queues, weights ride the gpsimd SWDGE queue. ---
    xT = [persist.tile([128, 1024], bf16, name=f"xT{t}") for t in range(2)]
    nc.sync.dma_start_transpose(xT[0][:], x[:, 0:128])
    nc.scalar.dma_start_transpose(xT[1][:], x[:, 128:256])
    pe_sb = [persist.tile([128, 1024], bf16, name=f"pe_sb{t}") for t in range(2)]
    nc.sync.dma_start(out=pe_sb[0][:], in_=petr[0])
    nc.scalar.dma_start(out=pe_sb[1][:], in_=petr[1])
    w_sb = {}
    for wname in ("wq", "wk", "wv"):
        w_sb[wname] = [
            persist.tile([128, 512], bf16, name=f"{wname}_sb{t}") for t in range(2)
        ]
    for t in range(2):
        nc.gpsimd.dma_start(out=w_sb["wk"][t][:], in_=wkr[t])
    for t in range(2):
        nc.gpsimd.dma_start(out=w_sb["wq"][t][:], in_=wqr[t])
    for t in range(2):
        nc.gpsimd.dma_start(out=w_sb["wv"][t][:], in_=wvr[t])
    bq_sb = persist.tile([128, 4], f32, name="bq_sb")
    nc.gpsimd.dma_start(out=bq_sb[:], in_=bqc[:, :])

    # qkT adds split per (t, l-half) so each fires as soon as its inputs land
    qkT = [persist.tile([128, 1024], bf16, name=f"qkT{t}") for t in range(2)]
    for g in range(2):
        for t in range(2):
            sl = slice(g * 512, (g + 1) * 512)
            nc.vector.tensor_add(qkT[t][:, sl], xT[t][:, sl], pe_sb[t][:, sl])

    # --- persistent SBUF operands ---
    kT2 = [persist.tile([128, 1024], bf16, name=f"kT2{j}") for j in range(4)]
    qT = [persist.tile([128, 1024], bf16, name=f"qT{j}") for j in range(4)]
    v_sb = [persist.tile([128, 8, 65], bf16, name=f"v_sb{m}") for m in range(8)]
    out_sb = persist.tile([128, 8, 512], f32, name="out_sb")

    # --- the PSUM chunk ring: 6 banks, [128, 512] f32 chunks ---
    s_ring = ring_pool.tile([128, RING_CHUNKS * 512], f32, name="s_ring")
    ring = {"pos": 0}

    def ring_chunk():
        p = ring["pos"]
        ring["pos"] = (p + 1) % RING_CHUNKS
        return s_ring[:, p * 512:(p + 1) * 512], p

    def ring_skip(n):
        ring["pos"] = (ring["pos"] + n) % RING_CHUNKS

    # --- exp stream: S-chunks queue up; every 3 chunks -> one FD=1536 exp ---
    epos = {}       # (h, mc, l2) -> (e_tile, col_offset)
    pending = []    # [(key, ring_pos)] of un-exp'd S chunks
    on_flush = {"hook": None}

    def flush_exp(force=False):
        if len(pending) < EXP_CHUNKS and not force:
            return
        n = len(pending)
        start = pending[0][1]
        assert start in (0, 3), f"misaligned exp read at ring pos {start}"
        for i, (_, p) in enumerate(pending):
            assert p == start + i
        e = epool.tile([128, EXP_CHUNKS * 512], bf16, tag="e", name="e")
        nc.scalar.activation(
            e[:, 0:n * 512], s_ring[:, start * 512:(start + n) * 512],
            Exp, scale=float(HD) ** -0.5,
        )
        done = []
        for i, (key, _) in enumerate(pending):
            epos[key] = (e, i * 512)
            done.append(key)
        pending.clear()
        if on_flush["hook"] is not None:
            on_flush["hook"](done)

    def s_pair(j, mc, l2):
        """Concurrent row-tiled S matmuls for heads 2j (rows 0:64) and
        2j+1 (rows 64:128); each writes one ring chunk."""
        msl = slice(mc * 128, (mc + 1) * 128)
        lsl = slice(l2 * 512, (l2 + 1) * 512)
        for half in range(2):
            c, p = ring_chunk()
            rows = slice(64 * half, 64 * half + 64)
            nc.tensor.matmul(
                c,
                lhsT=kT2[j][rows, msl],
                rhs=qT[j][rows, lsl],
                start=True,
                stop=True,
            )
            pending.append(((2 * j + half, mc, l2), p))
            if len(pending) == EXP_CHUNKS:
                flush_exp()

    # --- projections: each emits one ring chunk, drained by DVE ---
    def k_chunk(j, l2):
        c, _ = ring_chunk()
        for c2 in range(2):
            nc.tensor.matmul(
                c,
                lhsT=w_sb["wk"][c2][:, j * 128:(j + 1) * 128],
                rhs=qkT[c2][:, l2 * 512:(l2 + 1) * 512],
                start=(c2 == 0),
                stop=(c2 == 1),
            )
        nc.vector.tensor_copy(kT2[j][:, l2 * 512:(l2 + 1) * 512], c)

    def q_chunk(j, l2):
        c, _ = ring_chunk()
        for c2 in range(2):
            nc.tensor.matmul(
                c,
                lhsT=w_sb["wq"][c2][:, j * 128:(j + 1) * 128],
                rhs=qkT[c2][:, l2 * 512:(l2 + 1) * 512],
                start=(c2 == 0),
                stop=(c2 == 1),
            )
        nc.vector.tensor_scalar_add(
            qT[j][:, l2 * 512:(l2 + 1) * 512], c, bq_sb[:, j:j + 1]
        )

    def v_chunk(m):
        c, _ = ring_chunk()
        for c2 in range(2):
            nc.tensor.matmul(
                c,
                lhsT=xT[c2][:, m * 128:(m + 1) * 128],
                rhs=w_sb["wv"][c2][:],
                start=(c2 == 0),
                stop=(c2 == 1),
            )
        nc.vector.tensor_copy(
            v_sb[m][:, :, 0:64], c.rearrange("p (h d) -> p h d", h=8)
        )
        nc.vector.memset(v_sb[m][:, :, 64:65], 1.0)

    # --- O accumulation: quad (h, q) covers l-chunks 4q..4q+3.
    # NOTE: start=True clears has_written for the whole PSUM bank, so the
    # four column-group chains must run chain-sequential (g outer), never
    # interleaved -- an interleaved start would wipe the other chains'
    # first accumulation. ---
    oq = {}  # (h, q) -> {"pO": tile}

    def o_start(h, q):
        oq[(h, q)] = {
            "pO": o_ps.tile([128, 260], f32, tag="o", name="pO"),
        }

    def o_chains(h, q, mc_hold=None):
        """Emit the quad's 32 MMs chain-by-chain. mc_hold orders each
        chain's matmul on the given mc last (it is the chunk exp'd last)."""
        st = oq[(h, q)]
        mcs = list(range(8))
        if mc_hold is not None:
            mcs = [m for m in mcs if m != mc_hold] + [mc_hold]
        for g in range(4):
            for i, mc in enumerate(mcs):
                e, off = epos[(h, mc, q)]
                nc.tensor.matmul(
                    st["pO"][:, 65 * g:65 * g + 65],
                    lhsT=e[:, off + g * 128:off + (g + 1) * 128],
                    rhs=v_sb[mc][:, h, :],
                    start=(i == 0),
                    stop=(i == 7),
                )

    def o_finish(h, q, dma_engines=None):
        st = oq.pop((h, q))
        pO = st["pO"]
        hsl = slice(h * 64, (h + 1) * 64)
        pOr = pO.rearrange("p (g c) -> p g c", g=4)      # [128, 4, 65]
        rc = rpool.tile([128, 4], f32, tag="rc", name="rc")
        nc.vector.reciprocal(rc[:], pOr[:, :, 64])
        rcb = bass.AP(
            tensor=rc.tensor, offset=rc.offset,
            ap=[rc.ap[0], rc.ap[1], [0, 64]],
        )
        nc.vector.tensor_mul(
            out_sb[:, 4 * q:4 * q + 4, hsl], pOr[:, :, 0:64], rcb
        )
        eng = dma_engines or (nc.sync if (h + q) % 2 == 0 else nc.gpsimd)
        eng.dma_start(
            out=outr[:, 4 * q:4 * q + 4, hsl],
            in_=out_sb[:, 4 * q:4 * q + 4, hsl],
        )

    def o_quad(h, q):
        o_start(h, q)
        o_chains(h, q)
        o_finish(h, q)

    # ---------------- schedule ----------------
    # Head: K0 + Q0 projections (4 chunks + 2 skips keeps exp alignment:
    # the first S chunk lands at ring pos 0).
    k_chunk(0, 0)
    k_chunk(0, 1)
    q_chunk(0, 0)
    q_chunk(0, 1)
    ring_skip(2)

    # Projection triples may only be injected when `pending` is empty (so
    # they never split an exp's 3 contiguous chunks) and always allocate
    # exactly 3 ring positions (chunks + skips) to keep exp reads aligned
    # at ring pos 0/3. pending==0 happens after global S-pair p iff
    # p % 3 == 2; within group gi that is at pair indices
    # {2,5} / {0,3,6} / {1,4,7} for gi % 3 == 0 / 1 / 2.
    def triple_v(m0, m1, m2):
        def t():
            v_chunk(m0), v_chunk(m1), v_chunk(m2)
        return t

    def triple_v2s(m0, m1):
        def t():
            v_chunk(m0), v_chunk(m1), ring_skip(1)
        return t

    def triple_kkq(j):
        def t():
            k_chunk(j, 0), k_chunk(j, 1), q_chunk(j, 0)
        return t

    def triple_qss(j):
        def t():
            q_chunk(j, 1), ring_skip(2)
        return t

    blocks_plan = {  # (group_index, within_group_pair_index) -> thunk
        (0, 2): triple_v(0, 1, 2),
        (0, 5): triple_v(3, 4, 5),
        (1, 0): triple_v2s(6, 7),
        (1, 3): triple_kkq(1),
        (1, 6): triple_qss(1),
        (3, 2): triple_kkq(2),
        (3, 5): triple_qss(2),
        (5, 1): triple_kkq(3),
        (5, 4): triple_qss(3),
    }
    # O quads run one group behind their exps (e-tiles of group gi are
    # consumed during group gi+1); the final pair's q=1 quads are emitted
    # eagerly inside the last group so the post-exp tail stays short.
    quads_plan = {
        (1, 1): (0, 0), (1, 4): (1, 0),
        (2, 1): (0, 1), (2, 4): (1, 1),
        (3, 1): (2, 0), (3, 4): (3, 0),
        (4, 1): (2, 1), (4, 4): (3, 1),
        (5, 1): (4, 0), (5, 4): (5, 0),
        (6, 1): (4, 1), (6, 4): (5, 1),
    }

    groups = [(j, l2) for j in range(4) for l2 in range(2)]
    for gi, (j, l2) in enumerate(groups[:-1]):
        for mc in range(8):
            s_pair(j, mc, l2)
            blk = blocks_plan.get((gi, mc))
            if blk is not None:
                assert not pending, f"proj block at ({gi},{mc}) with pending"
                blk()
            hq = quads_plan.get((gi, mc))
            if hq is not None:
                o_quad(*hq)

    # last group (3, 1): retire (6,0)/(7,0) early, then emit the final two
    # quads with each chain's mc=7 matmul held last so only the held MMs
    # trail the final exp.
    j, l2 = groups[-1]
    s_pair(j, 0, l2)
    o_quad(6, 0)
    s_pair(j, 1, l2)
    o_quad(7, 0)
    for mc in range(2, 8):
        s_pair(j, mc, l2)
    if pending:
        flush_exp(force=True)
    o_start(6, 1)
    o_start(7, 1)
    o_chains(6, 1, mc_hold=7)
    o_chains(7, 1, mc_hold=7)
    o_finish(6, 1, dma_engines=nc.sync)
    o_finish(7, 1, dma_engines=nc.gpsimd)

    ctx.close()


def _build():
    if "nc" in _CACHE:
        return _CACHE["nc"]
    nc = bacc.Bacc("TRN2", target_bir_lowering=False, debug=False, num_devices=8)
    aps = {
        "x": nc.dram_tensor("x", [L, D], bf16, kind="ExternalInput").ap(),
        "wq": nc.dram_tensor("wq", [D, J], bf16, kind="ExternalInput").ap(),
        "wk": nc.dram_tensor("wk", [D, J], bf16, kind="ExternalInput").ap(),
        "wv": nc.dram_tensor("wv", [D, J], bf16, kind="ExternalInput").ap(),
        "bqc": nc.dram_tensor("bqc", [128, 4], f32, kind="ExternalInput").ap(),
        "pet": nc.dram_tensor("pet", [D, L], bf16, kind="ExternalInput").ap(),
        "out": nc.dram_tensor("out", [L, J], f32, kind="ExternalOutput").ap(),
    }
    with tile.TileContext(nc) as tc:
        _emit(tc, aps)
    nc.compile()
    _CACHE["nc"] = nc
    return nc


def _pe_T():
    embed = np.arange(L, dtype=np.float32)
    dim_t = np.arange(D, dtype=np.float32)
    dim_t = (np.float32(TEMPERATURE) ** (2.0 * np.floor(dim_t / 2.0) / np.float32(D))).astype(np.float32)
    pos = embed[:, None] / dim_t  # [L, D]
    pe = np.stack([np.sin(pos[:, 0::2]), np.cos(pos[:, 1::2])], axis=2).reshape(L, D)
    return np.ascontiguousarray(pe.T.astype(np.float32))  # [D, L]


def kernel(**inputs):
    global LAST_RESULT
    bf = np.float16
    x = np.asarray(inputs["x"], dtype=np.float32).astype(bf)
    wq = np.ascontiguousarray(np.asarray(inputs["Wq"], dtype=np.float32).astype(bf))
    wk = np.ascontiguousarray(np.asarray(inputs["Wk"], dtype=np.float32).astype(bf))
    wv = np.ascontiguousarray(np.asarray(inputs["Wv"], dtype=np.float32).astype(bf))
    bq = np.asarray(inputs["bq"], dtype=np.float32)
    bv = np.asarray(inputs["bv"], dtype=np.float32)

    nc = _build()
    bqc = np.ascontiguousarray(np.repeat(bq, HD).reshape(4, 128).T)  # [128, 4]
    pet = _pe_T().astype(bf)
    base = {"wq": wq, "wk": wk, "wv": wv, "bqc": bqc, "pet": pet}
    in_maps = [{**base, "x": np.ascontiguousarray(x[b])} for b in range(B)]
    res = run_bass_kernel_spmd(
        nc, in_maps, core_ids=list(range(B)), trace=TRACE
    )
    LAST_RESULT = res
    out = np.stack([res.results[b]["out"] for b in range(B)]).astype(np.float32)
    out += np.repeat(bv, HD)[None, None, :]
    return out


# revision 10
# speedup vs baseline: 1.2004x; 1.2004x over previous
"""Multi-head distance (attention) layer on 8 TRN2 NeuronCores.

Sharding: data-parallel over batch. B=8 -> one batch element per core.
Each core computes a full multi-head self-attention for its [L=1024, D=256]
slice with H=8 heads of dim 64. No collectives needed.

Per-core algorithm (fp16 matmul operands; ScalarE exp is the pacing engine):
  xT    = DMA-xbar transpose of x (HBM -> SBUF, no PE involvement)
  qkT   = xT + peT               (pos-enc, host-precomputed, DVE)
  qT    = Wq.T @ qkT + bq        per head-pair tiles [128(d), 1024(l)]
  kT2   = Wk.T @ qkT             per head-pair tiles [128(d), 1024(m)]
  v     = xT.T @ Wv              [m, 8, 64+1] tiles, ones column for Z
  S^T   = per-head kT2/qT matmuls, K=64, issued as CONCURRENT row-tiled
          pairs (heads 2j/2j+1 on PE row-groups 0-1/2-3 via base_partition
          0/64) -> 2x S throughput vs the zero-padded K=128 formulation.
  exp   = ScalarE, FD=1536 reads from a 6-bank PSUM ring ([128, 3072] f32)
          that all matmul chunks ([128, 512] = 1 bank) allocate through
          sequentially. Projection chunks interleave in mod-3 blocks (with
          ring-position skips) so every exp read is contiguous at ring
          offset 0 or 1536. 43 exp calls instead of 64 -> ~6.5us less
          ScalarE busy; ScalarE does nothing else.
  O     = e.T @ [v_h | 1] accumulated over 8 m-chunks into [128, 260] PSUM
          (2 banks double-buffered), normalized by DVE reciprocal+mul,
          DMA'd out per (head, quad). The final head-pair's q=1 quads are
          emitted eagerly per-exp so the tail after the last exp is short.
Bias handling: bq added during the Q PSUM drain (DVE, per-partition
scalar); bk only shifts score rows by a constant (softmax-invariant) so it
is dropped; bv shifts the output by exactly repeat(bv, 64) because softmax
rows sum to 1, added on the host.
"""

import numpy as np
import ml_dtypes

import concourse.bass as bass
import concourse.mybir as mybir
import concourse.tile as tile
from concourse import bacc
from concourse.bass_utils import run_bass_kernel_spmd

B, L, D = 8, 1024, 256
H, HD = 8, 64
J = H * HD  # 512
TEMPERATURE = 10000.0

f32 = mybir.dt.float32
bf16 = mybir.dt.float16  # fp16: same PE rate as bf16, 8x the mantissa

_CACHE = {}
LAST_RESULT = None  # BassKernelResults of the most recent run (for profiling)
TRACE = False

RING_CHUNKS = 6   # 6 PSUM banks of [128, 512] f32
EXP_CHUNKS = 3    # FD=1536 per ScalarE exp call


def _emit(tc, aps):
    nc = tc.nc
    Exp = mybir.ActivationFunctionType.Exp
    x, wq, wk, wv, bqc, pet, out = (
        aps["x"], aps["wq"], aps["wk"], aps["wv"], aps["bqc"], aps["pet"], aps["out"],
    )

    petr = pet.rearrange("(t p) l -> t p l", p=128)      # [2, 128, 1024]
    wqr = wq.rearrange("(t p) j -> t p j", p=128)        # [2, 128, 512]
    wkr = wk.rearrange("(t p) j -> t p j", p=128)
    wvr = wv.rearrange("(t p) j -> t p j", p=128)
    outr = out.rearrange("(n p) j -> p n j", p=128)      # [128, 8, 512]

    import contextlib
    ctx = contextlib.ExitStack()
    persist = ctx.enter_context(tc.tile_pool(name="persist", bufs=1))
    epool = ctx.enter_context(tc.tile_pool(name="epool", bufs=10))
    rpool = ctx.enter_context(tc.tile_pool(name="rpool", bufs=4))
    ring_pool = ctx.enter_context(tc.tile_pool(name="ringp", bufs=1, space="PSUM"))
    o_ps = ctx.enter_context(tc.tile_pool(name="ops", bufs=2, space="PSUM"))

    # --- ACT exp-table preload (off the attention critical path) ---
    sc_in = persist.tile([128, 8], f32, name="sc_in")
    sc_out = persist.tile([128, 8], f32, name="sc_out")
    nc.vector.memset(sc_in[:], 0.0)
    nc.scalar.activation(sc_out[:], sc_in[:], Exp)

    # --- input DMAs. xT via the DMA xbar transpose (no PE); sync + scalar
    # host the two HWDGE queues, weights ride the gpsimd SWDGE queue. ---
    xT = [persist.tile([128, 1024], bf16, name=f"xT{t}") for t in range(2)]
    nc.sync.dma_start_transpose(xT[0][:], x[:, 0:128])
    nc.scalar.dma_start_transpose(xT[1][:], x[:, 128:256])
    pe_sb = [persist.tile([128, 1024], bf16, name=f"pe_sb{t}") for t in range(2)]
    nc.sync.dma_start(out=pe_sb[0][:], in_=petr[0])
    nc.scalar.dma_start(out=pe_sb[1][:], in_=petr[1])
    w_sb = {}
    for wname in ("wq", "wk", "wv"):
        w_sb[wname] = [
            persist.tile([128, 512], bf16, name=f"{wname}_sb{t}") for t in range(2)
        ]
    for t in range(2):
        nc.gpsimd.dma_start(out=w_sb["wk"][t][:], in_=wkr[t])
    for t in range(2):
        nc.gpsimd.dma_start(out=w_sb["wq"][t][:], in_=wqr[t])
    for t in range(2):
        nc.gpsimd.dma_start(out=w_sb["wv"][t][:], in_=wvr[t])
    bq_sb = persist.tile([128, 4], f32, name="bq_sb")
    nc.gpsimd.dma_start(out=bq_sb[:], in_=bqc[:, :])

    # qkT adds split per (t, l-half) so each fires as soon as its inputs land
    qkT = [persist.tile([128, 1024], bf16, name=f"qkT{t}") for t in range(2)]
    for g in range(2):
        for t in range(2):
            sl = slice(g * 512, (g + 1) * 512)
            nc.vector.tensor_add(qkT[t][:, sl], xT[t][:, sl], pe_sb[t][:, sl])

    # --- persistent SBUF operands ---
    kT2 = [persist.tile([128, 1024], bf16, name=f"kT2{j}") for j in range(4)]
    qT = [persist.tile([128, 1024], bf16, name=f"qT{j}") for j in range(4)]
    v_sb = [persist.tile([128, 8, 65], bf16, name=f"v_sb{m}") for m in range(8)]
    out_sb = persist.tile([128, 8, 512], f32, name="out_sb")

    # --- the PSUM chunk ring: 6 banks, [128, 512] f32 chunks ---
    s_ring = ring_pool.tile([128, RING_CHUNKS * 512], f32, name="s_ring")
    ring = {"pos": 0}

    def ring_chunk():
        p = ring["pos"]
        ring["pos"] = (p + 1) % RING_CHUNKS
        return s_ring[:, p * 512:(p + 1) * 512], p

    def ring_skip(n):
        ring["pos"] = (ring["pos"] + n) % RING_CHUNKS

    # --- exp stream: S-chunks queue up; every 3 chunks -> one FD=1536 exp ---
    epos = {}       # (h, mc, l2) -> (e_tile, col_offset)
    pending = []    # [(key, ring_pos)] of un-exp'd S chunks
    on_flush = {"hook": None}

    def flush_exp(force=False):
        if len(pending) < EXP_CHUNKS and not force:
            return
        n = len(pending)
        start = pending[0][1]
        assert start in (0, 3), f"misaligned exp read at ring pos {start}"
        for i, (_, p) in enumerate(pending):
            assert p == start + i
        e = epool.tile([128, EXP_CHUNKS * 512], bf16, tag="e", name="e")
        nc.scalar.activation(
            e[:, 0:n * 512], s_ring[:, start * 512:(start + n) * 512],
            Exp, scale=float(HD) ** -0.5,
        )
        done = []
        for i, (key, _) in enumerate(pending):
            epos[key] = (e, i * 512)
            done.append(key)
        pending.clear()
        if on_flush["hook"] is not None:
            on_flush["hook"](done)

    def s_pair(j, mc, l2):
        """Concurrent row-tiled S matmuls for heads 2j (rows 0:64) and
        2j+1 (rows 64:128); each writes one ring chunk."""
        msl = slice(mc * 128, (mc + 1) * 128)
        lsl = slice(l2 * 512, (l2 + 1) * 512)
        for half in range(2):
            c, p = ring_chunk()
            rows = slice(64 * half, 64 * half + 64)
            nc.tensor.matmul(
                c,
                lhsT=kT2[j][rows, msl],
                rhs=qT[j][rows, lsl],
                start=True,
                stop=True,
            )
            pending.append(((2 * j + half, mc, l2), p))
            if len(pending) == EXP_CHUNKS:
                flush_exp()

    # --- projections: each emits one ring chunk, drained by DVE ---
    def k_chunk(j, l2):
        c, _ = ring_chunk()
        for c2 in range(2):
            nc.tensor.matmul(
                c,
                lhsT=w_sb["wk"][c2][:, j * 128:(j + 1) * 128],
                rhs=qkT[c2][:, l2 * 512:(l2 + 1) * 512],
                start=(c2 == 0),
                stop=(c2 == 1),
            )
        nc.vector.tensor_copy(kT2[j][:, l2 * 512:(l2 + 1) * 512], c)

    def q_chunk(j, l2):
        c, _ = ring_chunk()
        for c2 in range(2):
            nc.tensor.matmul(
                c,
                lhsT=w_sb["wq"][c2][:, j * 128:(j + 1) * 128],
                rhs=qkT[c2][:, l2 * 512:(l2 + 1) * 512],
                start=(c2 == 0),
                stop=(c2 == 1),
            )
        nc.vector.tensor_scalar_add(
            qT[j][:, l2 * 512:(l2 + 1) * 512], c, bq_sb[:, j:j + 1]
        )

    def v_chunk(m):
        c, _ = ring_chunk()
        for c2 in range(2):
            nc.tensor.matmul(
                c,
                lhsT=xT[c2][:, m * 128:(m + 1) * 128],
                rhs=w_sb["wv"][c2][:],
                start=(c2 == 0),
                stop=(c2 == 1),
            )
        nc.vector.tensor_copy(
            v_sb[m][:, :, 0:64], c.rearrange("p (h d) -> p h d", h=8)
        )
        nc.vector.memset(v_sb[m][:, :, 64:65], 1.0)

    # --- O accumulation: quad (h, q) covers l-chunks 4q..4q+3.
    # NOTE: start=True clears has_written for the whole PSUM bank, so the
    # four column-group chains must run chain-sequential (g outer), never
    # interleaved -- an interleaved start would wipe the other chains'
    # first accumulation. ---
    oq = {}  # (h, q) -> {"pO": tile}

    def o_start(h, q):
        oq[(h, q)] = {
            "pO": o_ps.tile([128, 260], f32, tag="o", name="pO"),
        }

    def o_chains(h, q, mc_hold=None):
        """Emit the quad's 32 MMs chain-by-chain. mc_hold orders each
        chain's matmul on the given mc last (it is the chunk exp'd last)."""
        st = oq[(h, q)]
        mcs = list(range(8))
        if mc_hold is not None:
            mcs = [m for m in mcs if m != mc_hold] + [mc_hold]
        for g in range(4):
            for i, mc in enumerate(mcs):
                e, off = epos[(h, mc, q)]
                nc.tensor.matmul(
                    st["pO"][:, 65 * g:65 * g + 65],
                    lhsT=e[:, off + g * 128:off + (g + 1) * 128],
                    rhs=v_sb[mc][:, h, :],
                    start=(i == 0),
                    stop=(i == 7),
                )

    # O work is emitted in 4-MM slices (half a column-group chain) so the
    # in-order PE queue never holds a long O burst in front of the S
    # matmuls that feed ScalarE -- a stalled exp stream lets the PE go
    # HAM-cold and everything doubles in cost.
    owork = []

    def enqueue_quads(hqs):
        slices = []
        for h, q in hqs:
            o_start(h, q)
            slices.append([(h, q, g, half) for g in range(4) for half in range(2)])
        for tup in zip(*slices):
            owork.extend(tup)

    def emit_slice():
        if not owork:
            return
        h, q, g, half = owork.pop(0)
        st = oq[(h, q)]
        for i in range(4):
            mc = 4 * half + i
            e, off = epos[(h, mc, q)]
            nc.tensor.matmul(
                st["pO"][:, 65 * g:65 * g + 65],
                lhsT=e[:, off + g * 128:off + (g + 1) * 128],
                rhs=v_sb[mc][:, h, :],
                start=(half == 0 and i == 0),
                stop=(half == 1 and i == 3),
            )
        if g == 3 and half == 1:
            o_finish(h, q)

    def o_finish(h, q, dma_engines=None):
        st = oq.pop((h, q))
        pO = st["pO"]
        hsl = slice(h * 64, (h + 1) * 64)
        pOr = pO.rearrange("p (g c) -> p g c", g=4)      # [128, 4, 65]
        rc = rpool.tile([128, 4], f32, tag="rc", name="rc")
        nc.vector.reciprocal(rc[:], pOr[:, :, 64])
        rcb = bass.AP(
            tensor=rc.tensor, offset=rc.offset,
            ap=[rc.ap[0], rc.ap[1], [0, 64]],
        )
        nc.vector.tensor_mul(
            out_sb[:, 4 * q:4 * q + 4, hsl], pOr[:, :, 0:64], rcb
        )
        eng = dma_engines or (nc.sync if (h + q) % 2 == 0 else nc.gpsimd)
        eng.dma_start(
            out=outr[:, 4 * q:4 * q + 4, hsl],
            in_=out_sb[:, 4 * q:4 * q + 4, hsl],
        )

    def o_quad(h, q):
        o_start(h, q)
        o_chains(h, q)
        o_finish(h, q)

    # ---------------- schedule ----------------
    # Head: K0 + Q0 projections (4 chunks + 2 skips keeps exp alignment:
    # the first S chunk lands at ring pos 0).
    k_chunk(0, 0)
    k_chunk(0, 1)
    q_chunk(0, 0)
    q_chunk(0, 1)
    ring_skip(2)

    # Projection triples may only be injected when `pending` is empty (so
    # they never split an exp's 3 contiguous chunks) and always allocate
    # exactly 3 ring positions (chunks + skips) to keep exp reads aligned
    # at ring pos 0/3. pending==0 happens after global S-pair p iff
    # p % 3 == 2; within group gi that is at pair indices
    # {2,5} / {0,3,6} / {1,4,7} for gi % 3 == 0 / 1 / 2.
    def triple_v(m0, m1, m2):
        def t():
            v_chunk(m0), v_chunk(m1), v_chunk(m2)
        return t

    def triple_v2s(m0, m1):
        def t():
            v_chunk(m0), v_chunk(m1), ring_skip(1)
        return t

    def triple_kkq(j):
        def t():
            k_chunk(j, 0), k_chunk(j, 1), q_chunk(j, 0)
        return t

    def triple_qss(j):
        def t():
            q_chunk(j, 1), ring_skip(2)
        return t

    blocks_plan = {  # (group_index, within_group_pair_index) -> thunk
        (0, 2): triple_v(0, 1, 2),
        (0, 5): triple_v(3, 4, 5),
        (1, 0): triple_v2s(6, 7),
        (1, 3): triple_kkq(1),
        (1, 6): triple_qss(1),
        (3, 2): triple_kkq(2),
        (3, 5): triple_qss(2),
        (5, 1): triple_kkq(3),
        (5, 4): triple_qss(3),
    }
    # O quads run one group behind their exps (e-tiles of group gi are
    # consumed during group gi+1); the final pair's q=1 quads are emitted
    # after the last group with each chain's mc=7 matmul held last so only
    # those trail the final exp.
    quads_plan = {
        1: [(0, 0), (1, 0)],
        2: [(0, 1), (1, 1)],
        3: [(2, 0), (3, 0)],
        4: [(2, 1), (3, 1)],
        5: [(4, 0), (5, 0)],
        6: [(4, 1), (5, 1)],
        7: [(6, 0), (7, 0)],
    }

    groups = [(j, l2) for j in range(4) for l2 in range(2)]
    for gi, (j, l2) in enumerate(groups):
        if gi in quads_plan:
            enqueue_quads(quads_plan[gi])
        for mc in range(8):
            s_pair(j, mc, l2)
            blk = blocks_plan.get((gi, mc))
            if blk is not None:
                assert not pending, f"proj block at ({gi},{mc}) with pending"
                blk()
            emit_slice()
            emit_slice()
        while owork:
            emit_slice()

    if pending:
        flush_exp(force=True)
    o_start(6, 1)
    o_start(7, 1)
    o_chains(6, 1, mc_hold=7)
    o_chains(7, 1, mc_hold=7)
    o_finish(6, 1, dma_engines=nc.sync)
    o_finish(7, 1, dma_engines=nc.gpsimd)

    ctx.close()


def _build():
    if "nc" in _CACHE:
        return _CACHE["nc"]
    nc = bacc.Bacc("TRN2", target_bir_lowering=False, debug=False, num_devices=8)
    aps = {
        "x": nc.dram_tensor("x", [L, D], bf16, kind="ExternalInput").ap(),
        "wq": nc.dram_tensor("wq", [D, J], bf16, kind="ExternalInput").ap(),
        "wk": nc.dram_tensor("wk", [D, J], bf16, kind="ExternalInput").ap(),
        "wv": nc.dram_tensor("wv", [D, J], bf16, kind="ExternalInput").ap(),
        "bqc": nc.dram_tensor("bqc", [128, 4], f32, kind="ExternalInput").ap(),
        "pet": nc.dram_tensor("pet", [D, L], bf16, kind="ExternalInput").ap(),
        "out": nc.dram_tensor("out", [L, J], f32, kind="ExternalOutput").ap(),
    }
    with tile.TileContext(nc) as tc:
        _emit(tc, aps)
    nc.compile()
    _CACHE["nc"] = nc
    return nc


def _pe_T():
    embed = np.arange(L, dtype=np.float32)
    dim_t = np.arange(D, dtype=np.float32)
    dim_t = (np.float32(TEMPERATURE) ** (2.0 * np.floor(dim_t / 2.0) / np.float32(D))).astype(np.float32)
    pos = embed[:, None] / dim_t  # [L, D]
    pe = np.stack([np.sin(pos[:, 0::2]), np.cos(pos[:, 1::2])], axis=2).reshape(L, D)
    return np.ascontiguousarray(pe.T.astype(np.float32))  # [D, L]


def kernel(**inputs):
    global LAST_RESULT
    bf = np.float16
    x = np.asarray(inputs["x"], dtype=np.float32).astype(bf)
    wq = np.ascontiguousarray(np.asarray(inputs["Wq"], dtype=np.float32).astype(bf))
    wk = np.ascontiguousarray(np.asarray(inputs["Wk"], dtype=np.float32).astype(bf))
    wv = np.ascontiguousarray(np.asarray(inputs["Wv"], dtype=np.float32).astype(bf))
    bq = np.asarray(inputs["bq"], dtype=np.float32)
    bv = np.asarray(inputs["bv"], dtype=np.float32)

    nc = _build()
    bqc = np.ascontiguousarray(np.repeat(bq, HD).reshape(4, 128).T)  # [128, 4]
    pet = _pe_T().astype(bf)
    base = {"wq": wq, "wk": wk, "wv": wv, "bqc": bqc, "pet": pet}
    in_maps = [{**base, "x": np.ascontiguousarray(x[b])} for b in range(B)]
    res = run_bass_kernel_spmd(
        nc, in_maps, core_ids=list(range(B)), trace=TRACE
    )
    LAST_RESULT = res
    out = np.stack([res.results[b]["out"] for b in range(B)]).astype(np.float32)
    out += np.repeat(bv, HD)[None, None, :]
    return out


# revision 15
# speedup vs baseline: 1.9408x; 1.6168x over previous
"""Multi-head distance (attention) layer on 8 TRN2 NeuronCores.

Sharding: data-parallel over batch. B=8 -> one batch element per core.
Each core computes a full multi-head self-attention for its [L=1024, D=256]
slice with H=8 heads of dim 64. No collectives needed.

Per-core algorithm (fp16 matmul operands; ScalarE exp is the pacing engine):
  xT    = DMA-xbar transpose of x (HBM -> SBUF, no PE involvement)
  qkT   = xT + peT               (pos-enc, host-precomputed, DVE)
  qT    = Wq.T @ qkT + bq        per head-pair tiles [128(d), 1024(l)]
  kT2   = Wk.T @ qkT             per head-pair tiles [128(d), 1024(m)]
  v     = xT.T @ Wv              [m, 8, 64+1] tiles, ones column for Z
  S^T   = per-head kT2/qT matmuls, K=64, issued as CONCURRENT row-tiled
          pairs (heads 2j/2j+1 on PE row-groups 0-1/2-3 via base_partition
          0/64) -> 2x S throughput vs the zero-padded K=128 formulation.
  exp   = ScalarE, FD=1536: S chunks fill two ping-pong PSUM "generation"
          tiles of [128, 1536] (3 banks each; PSUM dep tracking is
          whole-tile, so each generation is its own pool tile) and each
          full generation is exp'd in one ACT call. 43 exp calls instead
          of 64 -> ~6us less ScalarE busy; ScalarE does nothing else.
          Projection chunks ride exp-free generations of the same pool.
  O     = e.T @ [v_h | 1] accumulated over 8 m-chunks into [128, 260] PSUM
          (2 banks double-buffered), normalized by DVE reciprocal+mul,
          DMA'd out per (head, quad). The final head-pair's q=1 quads are
          emitted eagerly per-exp so the tail after the last exp is short.
Bias handling: bq added during the Q PSUM drain (DVE, per-partition
scalar); bk only shifts score rows by a constant (softmax-invariant) so it
is dropped; bv shifts the output by exactly repeat(bv, 64) because softmax
rows sum to 1, added on the host.
"""

import numpy as np
import ml_dtypes

import concourse.bass as bass
import concourse.mybir as mybir
import concourse.tile as tile
from concourse import bacc
from concourse.bass_utils import run_bass_kernel_spmd

B, L, D = 8, 1024, 256
H, HD = 8, 64
J = H * HD  # 512
TEMPERATURE = 10000.0

f32 = mybir.dt.float32
bf16 = mybir.dt.float16  # fp16: same PE rate as bf16, 8x the mantissa

_CACHE = {}
LAST_RESULT = None  # BassKernelResults of the most recent run (for profiling)
TRACE = False

EXP_CHUNKS = 3    # FD=1536 per ScalarE exp call


def _emit(tc, aps):
    nc = tc.nc
    Exp = mybir.ActivationFunctionType.Exp
    x, wq, wk, wv, bqc, pet, out = (
        aps["x"], aps["wq"], aps["wk"], aps["wv"], aps["bqc"], aps["pet"], aps["out"],
    )

    petr = pet.rearrange("(t p) l -> t p l", p=128)      # [2, 128, 1024]
    wqr = wq.rearrange("(t p) j -> t p j", p=128)        # [2, 128, 512]
    wkr = wk.rearrange("(t p) j -> t p j", p=128)
    wvr = wv.rearrange("(t p) j -> t p j", p=128)
    outr = out.rearrange("(n p) j -> p n j", p=128)      # [128, 8, 512]

    import contextlib
    ctx = contextlib.ExitStack()
    persist = ctx.enter_context(tc.tile_pool(name="persist", bufs=1))
    epool = ctx.enter_context(tc.tile_pool(name="epool", bufs=10))
    rpool = ctx.enter_context(tc.tile_pool(name="rpool", bufs=4))
    s_ps = ctx.enter_context(tc.tile_pool(name="sps", bufs=2, space="PSUM"))
    o_ps = ctx.enter_context(tc.tile_pool(name="ops", bufs=2, space="PSUM"))

    # --- ACT exp-table preload (off the attention critical path) ---
    sc_in = persist.tile([128, 8], f32, name="sc_in")
    sc_out = persist.tile([128, 8], f32, name="sc_out")
    nc.vector.memset(sc_in[:], 0.0)
    nc.scalar.activation(sc_out[:], sc_in[:], Exp)

    # --- input DMAs. xT via the DMA xbar transpose (no PE); sync + scalar
    # host the two HWDGE queues, weights ride the gpsimd SWDGE queue. ---
    xT = [persist.tile([128, 1024], bf16, name=f"xT{t}") for t in range(2)]
    nc.sync.dma_start_transpose(xT[0][:], x[:, 0:128])
    nc.scalar.dma_start_transpose(xT[1][:], x[:, 128:256])
    pe_sb = [persist.tile([128, 1024], bf16, name=f"pe_sb{t}") for t in range(2)]
    nc.sync.dma_start(out=pe_sb[0][:], in_=petr[0])
    nc.scalar.dma_start(out=pe_sb[1][:], in_=petr[1])
    w_sb = {}
    for wname in ("wq", "wk", "wv"):
        w_sb[wname] = [
            persist.tile([128, 512], bf16, name=f"{wname}_sb{t}") for t in range(2)
        ]
    for t in range(2):
        nc.gpsimd.dma_start(out=w_sb["wk"][t][:], in_=wkr[t])
    for t in range(2):
        nc.gpsimd.dma_start(out=w_sb["wq"][t][:], in_=wqr[t])
    for t in range(2):
        nc.gpsimd.dma_start(out=w_sb["wv"][t][:], in_=wvr[t])
    bq_sb = persist.tile([128, 4], f32, name="bq_sb")
    nc.gpsimd.dma_start(out=bq_sb[:], in_=bqc[:, :])

    # qkT adds split per (t, l-half) so each fires as soon as its inputs land
    qkT = [persist.tile([128, 1024], bf16, name=f"qkT{t}") for t in range(2)]
    for g in range(2):
        for t in range(2):
            sl = slice(g * 512, (g + 1) * 512)
            nc.vector.tensor_add(qkT[t][:, sl], xT[t][:, sl], pe_sb[t][:, sl])

    # --- persistent SBUF operands ---
    kT2 = [persist.tile([128, 1024], bf16, name=f"kT2{j}") for j in range(4)]
    qT = [persist.tile([128, 1024], bf16, name=f"qT{j}") for j in range(4)]
    v_sb = [persist.tile([128, 8, 65], bf16, name=f"v_sb{m}") for m in range(8)]
    out_sb = persist.tile([128, 8, 512], f32, name="out_sb")

    # --- PSUM generations: two ping-pong tiles of [128, 1536] (3 banks
    # each). PSUM dependency tracking is whole-tile, so each generation
    # (up to 3 matmul chunks + one whole-tile exp read) gets its own pool
    # tile; the pool's 2-slot rotation gives exactly one exp of pipeline
    # slack. Projection chunks use their own (exp-free) generations. ---
    gen = {"tile": None, "n": 0, "keys": []}

    def gen_chunk():
        if gen["tile"] is None:
            gen["tile"] = s_ps.tile([128, EXP_CHUNKS * 512], f32, tag="s", name="sg")
        n = gen["n"]
        gen["n"] = n + 1
        return gen["tile"][:, n * 512:(n + 1) * 512], n

    epos = {}       # (h, mc, l2) -> (e_tile, col_offset)

    def flush_exp(force=False):
        n = gen["n"]
        if n < EXP_CHUNKS and not force:
            return
        assert len(gen["keys"]) == n, "flush of a gen with projection chunks"
        e = epool.tile([128, EXP_CHUNKS * 512], bf16, tag="e", name="e")
        nc.scalar.activation(
            e[:, 0:n * 512], gen["tile"][:, 0:n * 512],
            Exp, scale=float(HD) ** -0.5,
        )
        for i, key in enumerate(gen["keys"]):
            epos[key] = (e, i * 512)
        gen["tile"] = None
        gen["n"] = 0
        gen["keys"] = []

    def close_proj_gen():
        assert not gen["keys"], "proj gen mixed with S chunks"
        gen["tile"] = None
        gen["n"] = 0

    def s_pair(j, mc, l2):
        """Concurrent row-tiled S matmuls for heads 2j (rows 0:64) and
        2j+1 (rows 64:128); each writes one generation chunk."""
        msl = slice(mc * 128, (mc + 1) * 128)
        lsl = slice(l2 * 512, (l2 + 1) * 512)
        for half in range(2):
            c, _ = gen_chunk()
            rows = slice(64 * half, 64 * half + 64)
            nc.tensor.matmul(
                c,
                lhsT=kT2[j][rows, msl],
                rhs=qT[j][rows, lsl],
                start=True,
                stop=True,
            )
            gen["keys"].append((2 * j + half, mc, l2))
            flush_exp()

    # --- projections: each emits one generation chunk, drained by DVE ---
    def k_chunk(j, l2):
        c, _ = gen_chunk()
        for c2 in range(2):
            nc.tensor.matmul(
                c,
                lhsT=w_sb["wk"][c2][:, j * 128:(j + 1) * 128],
                rhs=qkT[c2][:, l2 * 512:(l2 + 1) * 512],
                start=(c2 == 0),
                stop=(c2 == 1),
            )
        nc.vector.tensor_copy(kT2[j][:, l2 * 512:(l2 + 1) * 512], c)

    def q_chunk(j, l2):
        c, _ = gen_chunk()
        for c2 in range(2):
            nc.tensor.matmul(
                c,
                lhsT=w_sb["wq"][c2][:, j * 128:(j + 1) * 128],
                rhs=qkT[c2][:, l2 * 512:(l2 + 1) * 512],
                start=(c2 == 0),
                stop=(c2 == 1),
            )
        nc.vector.tensor_scalar_add(
            qT[j][:, l2 * 512:(l2 + 1) * 512], c, bq_sb[:, j:j + 1]
        )

    def v_chunk(m):
        c, _ = gen_chunk()
        for c2 in range(2):
            nc.tensor.matmul(
                c,
                lhsT=xT[c2][:, m * 128:(m + 1) * 128],
                rhs=w_sb["wv"][c2][:],
                start=(c2 == 0),
                stop=(c2 == 1),
            )
        nc.vector.tensor_copy(
            v_sb[m][:, :, 0:64], c.rearrange("p (h d) -> p h d", h=8)
        )
        nc.vector.memset(v_sb[m][:, :, 64:65], 1.0)

    # --- O accumulation: quad (h, q) covers l-chunks 4q..4q+3.
    # NOTE: start=True clears has_written for the whole PSUM bank, so the
    # four column-group chains must run chain-sequential (g outer), never
    # interleaved -- an interleaved start would wipe the other chains'
    # first accumulation. ---
    oq = {}  # (h, q) -> {"pO": tile}

    def o_start(h, q):
        oq[(h, q)] = {
            "pO": o_ps.tile([128, 260], f32, tag="o", name="pO"),
        }

    def o_chains(h, q, mc_hold=None):
        """Emit the quad's 32 MMs chain-by-chain. mc_hold orders each
        chain's matmul on the given mc last (it is the chunk exp'd last)."""
        st = oq[(h, q)]
        mcs = list(range(8))
        if mc_hold is not None:
            mcs = [m for m in mcs if m != mc_hold] + [mc_hold]
        for g in range(4):
            for i, mc in enumerate(mcs):
                e, off = epos[(h, mc, q)]
                nc.tensor.matmul(
                    st["pO"][:, 65 * g:65 * g + 65],
                    lhsT=e[:, off + g * 128:off + (g + 1) * 128],
                    rhs=v_sb[mc][:, h, :],
                    start=(i == 0),
                    stop=(i == 7),
                )

    # O work is emitted in 4-MM slices (half a column-group chain) so the
    # in-order PE queue never holds a long O burst in front of the S
    # matmuls that feed ScalarE -- a stalled exp stream lets the PE go
    # HAM-cold and everything doubles in cost.
    owork = []

    def enqueue_quads(hqs):
        slices = []
        for h, q in hqs:
            o_start(h, q)
            slices.append([(h, q, g, half) for g in range(4) for half in range(2)])
        for tup in zip(*slices):
            owork.extend(tup)

    def emit_slice():
        if not owork:
            return
        h, q, g, half = owork.pop(0)
        st = oq[(h, q)]
        for i in range(4):
            mc = 4 * half + i
            e, off = epos[(h, mc, q)]
            nc.tensor.matmul(
                st["pO"][:, 65 * g:65 * g + 65],
                lhsT=e[:, off + g * 128:off + (g + 1) * 128],
                rhs=v_sb[mc][:, h, :],
                start=(half == 0 and i == 0),
                stop=(half == 1 and i == 3),
            )
        if g == 3 and half == 1:
            o_finish(h, q)

    def o_finish(h, q, dma_engines=None):
        st = oq.pop((h, q))
        pO = st["pO"]
        hsl = slice(h * 64, (h + 1) * 64)
        pOr = pO.rearrange("p (g c) -> p g c", g=4)      # [128, 4, 65]
        rc = rpool.tile([128, 4], f32, tag="rc", name="rc")
        nc.vector.reciprocal(rc[:], pOr[:, :, 64])
        rcb = bass.AP(
            tensor=rc.tensor, offset=rc.offset,
            ap=[rc.ap[0], rc.ap[1], [0, 64]],
        )
        nc.vector.tensor_mul(
            out_sb[:, 4 * q:4 * q + 4, hsl], pOr[:, :, 0:64], rcb
        )
        eng = dma_engines or (nc.sync if (h + q) % 2 == 0 else nc.gpsimd)
        eng.dma_start(
            out=outr[:, 4 * q:4 * q + 4, hsl],
            in_=out_sb[:, 4 * q:4 * q + 4, hsl],
        )

    def o_quad(h, q):
        o_start(h, q)
        o_chains(h, q)
        o_finish(h, q)

    # ---------------- schedule ----------------
    # Head: K0 + Q0 + first V projections fill the first two generations.
    k_chunk(0, 0)
    k_chunk(0, 1)
    q_chunk(0, 0)
    close_proj_gen()
    q_chunk(0, 1)
    v_chunk(0)
    v_chunk(1)
    close_proj_gen()

    # Projection generations are injected at generation boundaries
    # (gen empty after global S-pair p iff p % 3 == 2; within group gi
    # that is pair index {2,5} / {0,3,6} / {1,4,7} for gi % 3 == 0/1/2).
    def proj_gen(*thunks):
        def t():
            for th in thunks:
                th()
            close_proj_gen()
        return t

    def K(j, l2):
        return lambda: k_chunk(j, l2)

    def Q(j, l2):
        return lambda: q_chunk(j, l2)

    def V(m):
        return lambda: v_chunk(m)

    blocks_plan = {  # (group_index, within_group_pair_index) -> thunk
        (0, 2): proj_gen(V(2), V(3), V(4)),
        (0, 5): proj_gen(V(5), V(6), V(7)),
        (1, 0): proj_gen(K(1, 0), K(1, 1), Q(1, 0)),
        (1, 3): proj_gen(Q(1, 1), K(2, 0), K(2, 1)),
        (1, 6): proj_gen(Q(2, 0), Q(2, 1), K(3, 0)),
        (2, 1): proj_gen(K(3, 1), Q(3, 0), Q(3, 1)),
    }
    # O quads run one group behind their exps (e-tiles of group gi are
    # consumed during group gi+1); the final pair's q=1 quads are emitted
    # after the last group with each chain's mc=7 matmul held last so only
    # those trail the final exp.
    quads_plan = {
        1: [(0, 0), (1, 0)],
        2: [(0, 1), (1, 1)],
        3: [(2, 0), (3, 0)],
        4: [(2, 1), (3, 1)],
        5: [(4, 0), (5, 0)],
        6: [(4, 1), (5, 1)],
        7: [(6, 0), (7, 0)],
    }

    groups = [(j, l2) for j in range(4) for l2 in range(2)]
    for gi, (j, l2) in enumerate(groups):
        if gi in quads_plan:
            enqueue_quads(quads_plan[gi])
        for mc in range(8):
            s_pair(j, mc, l2)
            blk = blocks_plan.get((gi, mc))
            if blk is not None:
                assert gen["n"] == 0, f"proj block at ({gi},{mc}) mid-gen"
                blk()
            emit_slice()
            emit_slice()
        while owork:
            emit_slice()

    if gen["n"]:
        flush_exp(force=True)
    o_start(6, 1)
    o_start(7, 1)
    o_chains(6, 1, mc_hold=7)
    o_chains(7, 1, mc_hold=7)
    o_finish(6, 1, dma_engines=nc.sync)
    o_finish(7, 1, dma_engines=nc.gpsimd)

    ctx.close()


def _build():
    if "nc" in _CACHE:
        return _CACHE["nc"]
    nc = bacc.Bacc("TRN2", target_bir_lowering=False, debug=False, num_devices=8)
    aps = {
        "x": nc.dram_tensor("x", [L, D], bf16, kind="ExternalInput").ap(),
        "wq": nc.dram_tensor("wq", [D, J], bf16, kind="ExternalInput").ap(),
        "wk": nc.dram_tensor("wk", [D, J], bf16, kind="ExternalInput").ap(),
        "wv": nc.dram_tensor("wv", [D, J], bf16, kind="ExternalInput").ap(),
        "bqc": nc.dram_tensor("bqc", [128, 4], f32, kind="ExternalInput").ap(),
        "pet": nc.dram_tensor("pet", [D, L], bf16, kind="ExternalInput").ap(),
        "out": nc.dram_tensor("out", [L, J], f32, kind="ExternalOutput").ap(),
    }
    with tile.TileContext(nc) as tc:
        _emit(tc, aps)
    nc.compile()
    _CACHE["nc"] = nc
    return nc


def _pe_T():
    embed = np.arange(L, dtype=np.float32)
    dim_t = np.arange(D, dtype=np.float32)
    dim_t = (np.float32(TEMPERATURE) ** (2.0 * np.floor(dim_t / 2.0) / np.float32(D))).astype(np.float32)
    pos = embed[:, None] / dim_t  # [L, D]
    pe = np.stack([np.sin(pos[:, 0::2]), np.cos(pos[:, 1::2])], axis=2).reshape(L, D)
    return np.ascontiguousarray(pe.T.astype(np.float32))  # [D, L]


def kernel(**inputs):
    global LAST_RESULT
    bf = np.float16
    x = np.asarray(inputs["x"], dtype=np.float32).astype(bf)
    wq = np.ascontiguousarray(np.asarray(inputs["Wq"], dtype=np.float32).astype(bf))
    wk = np.ascontiguousarray(np.asarray(inputs["Wk"], dtype=np.float32).astype(bf))
    wv = np.ascontiguousarray(np.asarray(inputs["Wv"], dtype=np.float32).astype(bf))
    bq = np.asarray(inputs["bq"], dtype=np.float32)
    bv = np.asarray(inputs["bv"], dtype=np.float32)

    nc = _build()
    bqc = np.ascontiguousarray(np.repeat(bq, HD).reshape(4, 128).T)  # [128, 4]
    pet = _pe_T().astype(bf)
    base = {"wq": wq, "wk": wk, "wv": wv, "bqc": bqc, "pet": pet}
    in_maps = [{**base, "x": np.ascontiguousarray(x[b])} for b in range(B)]
    res = run_bass_kernel_spmd(
        nc, in_maps, core_ids=list(range(B)), trace=TRACE
    )
    LAST_RESULT = res
    out = np.stack([res.results[b]["out"] for b in range(B)]).astype(np.float32)
    out += np.repeat(bv, HD)[None, None, :]
    return out


# revision 16
# speedup vs baseline: 1.9916x; 1.0262x over previous
"""Multi-head distance (attention) layer on 8 TRN2 NeuronCores.

Sharding: data-parallel over batch. B=8 -> one batch element per core.
Each core computes a full multi-head self-attention for its [L=1024, D=256]
slice with H=8 heads of dim 64. No collectives needed.

Per-core algorithm (fp16 matmul operands; ScalarE exp is the pacing engine):
  xT    = transpose(x) on the PE (vs identity; also warms the HAM clock
          gate early), drains split between ScalarE and DVE
  qkT   = xT + peT               (pos-enc, host-precomputed, DVE)
  qT    = Wq.T @ qkT + bq        per head-pair tiles [128(d), 1024(l)]
  kT2   = Wk.T @ qkT             per head-pair tiles [128(d), 1024(m)]
  v     = xT.T @ Wv              [m, 8, 64+1] tiles, ones column for Z
  S^T   = per-head kT2/qT matmuls, K=64, issued as CONCURRENT row-tiled
          pairs (heads 2j/2j+1 on PE row-groups 0-1/2-3 via base_partition
          0/64) -> 2x S throughput vs a zero-padded K=128 formulation.
  exp   = ScalarE, FD=1536: S chunks fill two ping-pong PSUM "generation"
          tiles of [128, 1536] (3 banks each; PSUM dep tracking is
          whole-tile, so each generation is its own pool tile) and each
          full generation is exp'd in one ACT call; ScalarE does nothing
          else mid-stream. K/Q projections for j>=1 ride every other
          generation as a [proj, S, S] mix (exp reads the S suffix) so the
          exp stream never bubbles. K0/Q0/V projections pipeline through
          the O-PSUM banks, which are idle until the first O quad.
  O     = e.T @ [v_h | 1] accumulated over 8 m-chunks into [128, 260] PSUM
          (2 banks double-buffered), emitted as 4-MM chain-slices
          interleaved two per S-pair (a long O burst in front of the
          in-order PE queue would stall the exp stream and let the PE go
          HAM-cold). Chains are strictly bank-sequential because a
          matmul's start=True clears has_written for the whole bank.
          Normalized by DVE reciprocal + broadcast multiply, DMA'd out per
          (head, quad).
Bias handling: bq added during the Q PSUM drain (DVE, per-partition
scalar); bk only shifts each score row by a constant (softmax-invariant)
so it is dropped; bv shifts the output by exactly repeat(bv, 64) because
softmax rows sum to 1, added on the host.
"""

import numpy as np
import ml_dtypes

import concourse.bass as bass
import concourse.mybir as mybir
import concourse.tile as tile
from concourse import bacc
from concourse.bass_utils import run_bass_kernel_spmd
from concourse.masks import make_identity

B, L, D = 8, 1024, 256
H, HD = 8, 64
J = H * HD  # 512
TEMPERATURE = 10000.0

f32 = mybir.dt.float32
bf16 = mybir.dt.float16  # fp16: same PE rate as bf16, 8x the mantissa

_CACHE = {}
LAST_RESULT = None  # BassKernelResults of the most recent run (for profiling)
TRACE = False

EXP_CHUNKS = 3    # FD=1536 per full ScalarE exp call


def _emit(tc, aps):
    nc = tc.nc
    Exp = mybir.ActivationFunctionType.Exp
    Copy = mybir.ActivationFunctionType.Copy
    x, wq, wk, wv, bqc, pet, out = (
        aps["x"], aps["wq"], aps["wk"], aps["wv"], aps["bqc"], aps["pet"], aps["out"],
    )

    xr = x.rearrange("(n p) c -> p n c", p=128)          # [128, 8, 256]
    petr = pet.rearrange("(t p) l -> t p l", p=128)      # [2, 128, 1024]
    wqr = wq.rearrange("(t p) j -> t p j", p=128)        # [2, 128, 512]
    wkr = wk.rearrange("(t p) j -> t p j", p=128)
    wvr = wv.rearrange("(t p) j -> t p j", p=128)
    outr = out.rearrange("(n p) j -> p n j", p=128)      # [128, 8, 512]

    import contextlib
    ctx = contextlib.ExitStack()
    persist = ctx.enter_context(tc.tile_pool(name="persist", bufs=1))
    epool = ctx.enter_context(tc.tile_pool(name="epool", bufs=10))
    rpool = ctx.enter_context(tc.tile_pool(name="rpool", bufs=4))
    s_ps = ctx.enter_context(tc.tile_pool(name="sps", bufs=2, space="PSUM"))
    o_ps = ctx.enter_context(tc.tile_pool(name="ops", bufs=2, space="PSUM"))

    # --- ACT exp-table preload (off the attention critical path) ---
    sc_in = persist.tile([128, 8], f32, name="sc_in")
    sc_out = persist.tile([128, 8], f32, name="sc_out")
    nc.vector.memset(sc_in[:], 0.0)
    nc.scalar.activation(sc_out[:], sc_in[:], Exp)

    # --- input DMAs: x on the sync HWDGE queue, pe on the scalar HWDGE
    # queue (ScalarE is idle until the first exp), weights on gpsimd ---
    x_sb = persist.tile([128, 8, 256], bf16, name="x_sb")
    for qtr in range(4):
        nc.sync.dma_start(out=x_sb[:, qtr * 2:(qtr + 1) * 2, :],
                          in_=xr[:, qtr * 2:(qtr + 1) * 2, :])
    pe_sb = [persist.tile([128, 1024], bf16, name=f"pe_sb{t}") for t in range(2)]
    for t in range(2):
        nc.scalar.dma_start(out=pe_sb[t][:], in_=petr[t])
    w_sb = {}
    for wname in ("wq", "wk", "wv"):
        w_sb[wname] = [
            persist.tile([128, 512], bf16, name=f"{wname}_sb{t}") for t in range(2)
        ]
    for t in range(2):
        nc.gpsimd.dma_start(out=w_sb["wk"][t][:], in_=wkr[t])
    for t in range(2):
        nc.gpsimd.dma_start(out=w_sb["wq"][t][:], in_=wqr[t])
    for t in range(2):
        nc.gpsimd.dma_start(out=w_sb["wv"][t][:], in_=wvr[t])
    bq_sb = persist.tile([128, 4], f32, name="bq_sb")
    nc.gpsimd.dma_start(out=bq_sb[:], in_=bqc[:, :])

    ident = persist.tile([128, 128], bf16, name="ident")
    make_identity(nc, ident)

    # --- transpose x via PE; one generation tile per xT half, drains
    # split between ScalarE and DVE ---
    xT = [persist.tile([128, 1024], bf16, name=f"xT{t}") for t in range(2)]
    for c2 in range(2):
        tp = s_ps.tile([128, EXP_CHUNKS * 512], f32, tag="s", name="tp")
        for n in range(8):
            nc.tensor.matmul(
                tp[:, n * 128:(n + 1) * 128],
                lhsT=x_sb[:, n, c2 * 128:(c2 + 1) * 128],
                rhs=ident[:],
                start=True,
                stop=True,
            )
        nc.scalar.activation(xT[c2][:, 0:512], tp[:, 0:512], Copy)
        nc.vector.tensor_copy(xT[c2][:, 512:1024], tp[:, 512:1024])

    # qkT adds split per (t, l-half) so each fires as soon as its
    # transpose-drain half lands
    qkT = [persist.tile([128, 1024], bf16, name=f"qkT{t}") for t in range(2)]
    for g in range(2):
        for t in range(2):
            sl = slice(g * 512, (g + 1) * 512)
            nc.vector.tensor_add(qkT[t][:, sl], xT[t][:, sl], pe_sb[t][:, sl])

    # --- persistent SBUF operands ---
    kT2 = [persist.tile([128, 1024], bf16, name=f"kT2{j}") for j in range(4)]
    qT = [persist.tile([128, 1024], bf16, name=f"qT{j}") for j in range(4)]
    v_sb = [persist.tile([128, 8, 65], bf16, name=f"v_sb{m}") for m in range(8)]
    out_sb = persist.tile([128, 8, 512], f32, name="out_sb")

    # --- projection chunk bodies (target PSUM AP supplied by the host
    # slot: an o_ps tile early on, or a mixed generation's first chunk) ---
    def k_body(j, l2, c, eng):
        for c2 in range(2):
            nc.tensor.matmul(
                c,
                lhsT=w_sb["wk"][c2][:, j * 128:(j + 1) * 128],
                rhs=qkT[c2][:, l2 * 512:(l2 + 1) * 512],
                start=(c2 == 0),
                stop=(c2 == 1),
            )
        if eng is nc.scalar:
            nc.scalar.activation(kT2[j][:, l2 * 512:(l2 + 1) * 512], c, Copy)
        else:
            nc.vector.tensor_copy(kT2[j][:, l2 * 512:(l2 + 1) * 512], c)

    def q_body(j, l2, c, eng):
        for c2 in range(2):
            nc.tensor.matmul(
                c,
                lhsT=w_sb["wq"][c2][:, j * 128:(j + 1) * 128],
                rhs=qkT[c2][:, l2 * 512:(l2 + 1) * 512],
                start=(c2 == 0),
                stop=(c2 == 1),
            )
        nc.vector.tensor_scalar_add(
            qT[j][:, l2 * 512:(l2 + 1) * 512], c, bq_sb[:, j:j + 1]
        )

    def v_body(m, c, eng):
        for c2 in range(2):
            nc.tensor.matmul(
                c,
                lhsT=xT[c2][:, m * 128:(m + 1) * 128],
                rhs=w_sb["wv"][c2][:],
                start=(c2 == 0),
                stop=(c2 == 1),
            )
        nc.vector.tensor_copy(
            v_sb[m][:, :, 0:64], c.rearrange("p (h d) -> p h d", h=8)
        )
        nc.vector.memset(v_sb[m][:, :, 64:65], 1.0)

    def o_proj(body):
        c = o_ps.tile([128, 512], f32, tag="o", name="pc")
        body(c)

    # --- PSUM generations: two ping-pong tiles of [128, 1536]. PSUM dep
    # tracking is whole-tile, so each generation is its own pool tile; the
    # 2-slot rotation gives one exp of pipeline slack. Mixed generations
    # put one projection chunk FIRST (cols 0:512, drained by DVE), then
    # two S chunks; exp reads the S suffix. ---
    from collections import deque
    proj_sched = deque()   # bodies for K/Q (j >= 1), one per mixed gen
    gen = {"tile": None, "off": 0, "cap": EXP_CHUNKS, "keys": [], "pure": 0}

    def open_gen():
        gen["tile"] = s_ps.tile([128, EXP_CHUNKS * 512], f32, tag="s", name="sg")
        gen["keys"] = []
        if proj_sched and gen["pure"] >= 1:
            body = proj_sched.popleft()
            body(gen["tile"][:, 0:512])
            gen["off"] = 1
            gen["cap"] = EXP_CHUNKS - 1
            gen["pure"] = 0
        else:
            gen["off"] = 0
            gen["cap"] = EXP_CHUNKS
            gen["pure"] += 1

    def gen_chunk():
        if gen["tile"] is None:
            open_gen()
        n = gen["off"] + len(gen["keys"])
        return gen["tile"][:, n * 512:(n + 1) * 512]

    epos = {}       # (h, mc, l2) -> (e_tile, col_offset)

    def flush_exp(force=False):
        nS = len(gen["keys"])
        if nS < gen["cap"] and not force:
            return
        off = gen["off"]
        e = epool.tile([128, EXP_CHUNKS * 512], bf16, tag="e", name="e")
        nc.scalar.activation(
            e[:, 0:nS * 512], gen["tile"][:, off * 512:(off + nS) * 512],
            Exp, scale=float(HD) ** -0.5,
        )
        for i, key in enumerate(gen["keys"]):
            epos[key] = (e, i * 512)
        gen["tile"] = None
        gen["keys"] = []

    def s_pair(j, mc, l2):
        """Concurrent row-tiled S matmuls for heads 2j (rows 0:64) and
        2j+1 (rows 64:128); each writes one generation chunk."""
        msl = slice(mc * 128, (mc + 1) * 128)
        lsl = slice(l2 * 512, (l2 + 1) * 512)
        for half in range(2):
            c = gen_chunk()
            rows = slice(64 * half, 64 * half + 64)
            nc.tensor.matmul(
                c,
                lhsT=kT2[j][rows, msl],
                rhs=qT[j][rows, lsl],
                start=True,
                stop=True,
            )
            gen["keys"].append((2 * j + half, mc, l2))
            flush_exp()

    # --- O accumulation: quad (h, q) covers l-chunks 4q..4q+3.
    # start=True clears has_written for the whole PSUM bank, so the four
    # column-group chains must run chain-sequential per bank. ---
    oq = {}  # (h, q) -> {"pO": tile}

    def o_start(h, q):
        oq[(h, q)] = {
            "pO": o_ps.tile([128, 260], f32, tag="o", name="pO"),
        }

    def o_chains(h, q, mc_hold=None):
        """Emit the quad's 32 MMs chain-by-chain. mc_hold orders each
        chain's matmul on the given mc last (it is the chunk exp'd last)."""
        st = oq[(h, q)]
        mcs = list(range(8))
        if mc_hold is not None:
            mcs = [m for m in mcs if m != mc_hold] + [mc_hold]
        for g in range(4):
            for i, mc in enumerate(mcs):
                e, off = epos[(h, mc, q)]
                nc.tensor.matmul(
                    st["pO"][:, 65 * g:65 * g + 65],
                    lhsT=e[:, off + g * 128:off + (g + 1) * 128],
                    rhs=v_sb[mc][:, h, :],
                    start=(i == 0),
                    stop=(i == 7),
                )

    def o_finish(h, q, dma_engine=None):
        st = oq.pop((h, q))
        pO = st["pO"]
        hsl = slice(h * 64, (h + 1) * 64)
        pOr = pO.rearrange("p (g c) -> p g c", g=4)      # [128, 4, 65]
        rc = rpool.tile([128, 4], f32, tag="rc", name="rc")
        nc.vector.reciprocal(rc[:], pOr[:, :, 64])
        rcb = bass.AP(
            tensor=rc.tensor, offset=rc.offset,
            ap=[rc.ap[0], rc.ap[1], [0, 64]],
        )
        nc.vector.tensor_mul(
            out_sb[:, 4 * q:4 * q + 4, hsl], pOr[:, :, 0:64], rcb
        )
        eng = dma_engine or (nc.sync if (h + q) % 2 == 0 else nc.gpsimd)
        eng.dma_start(
            out=outr[:, 4 * q:4 * q + 4, hsl],
            in_=out_sb[:, 4 * q:4 * q + 4, hsl],
        )

    # O work is emitted in 4-MM slices (half a column-group chain) so the
    # in-order PE queue never holds a long O burst in front of the S
    # matmuls that feed ScalarE.
    owork = []

    def enqueue_quads(hqs):
        slices = []
        for h, q in hqs:
            o_start(h, q)
            slices.append([(h, q, g, half) for g in range(4) for half in range(2)])
        for tup in zip(*slices):
            owork.extend(tup)

    def emit_slice():
        if not owork:
            return
        h, q, g, half = owork.pop(0)
        st = oq[(h, q)]
        for i in range(4):
            mc = 4 * half + i
            e, off = epos[(h, mc, q)]
            nc.tensor.matmul(
                st["pO"][:, 65 * g:65 * g + 65],
                lhsT=e[:, off + g * 128:off + (g + 1) * 128],
                rhs=v_sb[mc][:, h, :],
                start=(half == 0 and i == 0),
                stop=(half == 1 and i == 3),
            )
        if g == 3 and half == 1:
            o_finish(h, q)

    # ---------------- schedule ----------------
    # Head: the two gating projections (K0 l2=0, Q0 l2=0) first on the two
    # free O-PSUM banks, then their l2=1 halves and the first V chunks.
    o_proj(lambda c: k_body(0, 0, c, nc.scalar))
    o_proj(lambda c: q_body(0, 0, c, nc.vector))
    o_proj(lambda c: k_body(0, 1, c, nc.scalar))
    o_proj(lambda c: q_body(0, 1, c, nc.vector))
    o_proj(lambda c: v_body(0, c, nc.vector))
    o_proj(lambda c: v_body(1, c, nc.vector))

    # K/Q for j >= 1 ride mixed generations, one chunk every other gen.
    for j in range(1, 4):
        proj_sched.append(lambda c, j=j: k_body(j, 0, c, nc.vector))
        proj_sched.append(lambda c, j=j: k_body(j, 1, c, nc.vector))
        proj_sched.append(lambda c, j=j: q_body(j, 0, c, nc.vector))
        proj_sched.append(lambda c, j=j: q_body(j, 1, c, nc.vector))

    # Remaining V projections pipeline through the O-PSUM banks during
    # group 0 (the first O quad is not enqueued until group 1).
    v_plan = {0: 2, 1: 3, 2: 4, 3: 5, 4: 6, 5: 7}  # pair-idx -> v chunk

    # O quads run one group behind their exps; the final pair's q=1 quads
    # are emitted after the last group with mc=7 held last.
    quads_plan = {
        1: [(0, 0), (1, 0)],
        2: [(0, 1), (1, 1)],
        3: [(2, 0), (3, 0)],
        4: [(2, 1), (3, 1)],
        5: [(4, 0), (5, 0)],
        6: [(4, 1), (5, 1)],
        7: [(6, 0), (7, 0)],
    }

    groups = [(j, l2) for j in range(4) for l2 in range(2)]
    for gi, (j, l2) in enumerate(groups):
        if gi in quads_plan:
            enqueue_quads(quads_plan[gi])
        for mc in range(8):
            s_pair(j, mc, l2)
            if gi == 0 and mc in v_plan:
                o_proj(lambda c, m=v_plan[mc]: v_body(m, c, nc.vector))
            emit_slice()
            emit_slice()
        while owork:
            emit_slice()

    if gen["keys"]:
        flush_exp(force=True)
    o_start(6, 1)
    o_start(7, 1)
    o_chains(6, 1, mc_hold=7)
    o_chains(7, 1, mc_hold=7)
    o_finish(6, 1, dma_engine=nc.sync)
    o_finish(7, 1, dma_engine=nc.gpsimd)

    ctx.close()


def _build():
    if "nc" in _CACHE:
        return _CACHE["nc"]
    nc = bacc.Bacc("TRN2", target_bir_lowering=False, debug=False, num_devices=8)
    aps = {
        "x": nc.dram_tensor("x", [L, D], bf16, kind="ExternalInput").ap(),
        "wq": nc.dram_tensor("wq", [D, J], bf16, kind="ExternalInput").ap(),
        "wk": nc.dram_tensor("wk", [D, J], bf16, kind="ExternalInput").ap(),
        "wv": nc.dram_tensor("wv", [D, J], bf16, kind="ExternalInput").ap(),
        "bqc": nc.dram_tensor("bqc", [128, 4], f32, kind="ExternalInput").ap(),
        "pet": nc.dram_tensor("pet", [D, L], bf16, kind="ExternalInput").ap(),
        "out": nc.dram_tensor("out", [L, J], f32, kind="ExternalOutput").ap(),
    }
    with tile.TileContext(nc) as tc:
        _emit(tc, aps)
    nc.compile()
    _CACHE["nc"] = nc
    return nc


def _pe_T():
    embed = np.arange(L, dtype=np.float32)
    dim_t = np.arange(D, dtype=np.float32)
    dim_t = (np.float32(TEMPERATURE) ** (2.0 * np.floor(dim_t / 2.0) / np.float32(D))).astype(np.float32)
    pos = embed[:, None] / dim_t  # [L, D]
    pe = np.stack([np.sin(pos[:, 0::2]), np.cos(pos[:, 1::2])], axis=2).reshape(L, D)
    return np.ascontiguousarray(pe.T.astype(np.float32))  # [D, L]


def kernel(**inputs):
    global LAST_RESULT
    bf = np.float16
    x = np.asarray(inputs["x"], dtype=np.float32).astype(bf)
    wq = np.ascontiguousarray(np.asarray(inputs["Wq"], dtype=np.float32).astype(bf))
    wk = np.ascontiguousarray(np.asarray(inputs["Wk"], dtype=np.float32).astype(bf))
    wv = np.ascontiguousarray(np.asarray(inputs["Wv"], dtype=np.float32).astype(bf))
    bq = np.asarray(inputs["bq"], dtype=np.float32)
    bv = np.asarray(inputs["bv"], dtype=np.float32)

    nc = _build()
    bqc = np.ascontiguousarray(np.repeat(bq, HD).reshape(4, 128).T)  # [128, 4]
    pet = _pe_T().astype(bf)
    base = {"wq": wq, "wk": wk, "wv": wv, "bqc": bqc, "pet": pet}
    in_maps = [{**base, "x": np.ascontiguousarray(x[b])} for b in range(B)]
    res = run_bass_kernel_spmd(
        nc, in_maps, core_ids=list(range(B)), trace=TRACE
    )
    LAST_RESULT = res
    out = np.stack([res.results[b]["out"] for b in range(B)]).astype(np.float32)
    out += np.repeat(bv, HD)[None, None, :]
    return out


# revision 21
# speedup vs baseline: 2.0815x; 1.0451x over previous
"""Multi-head distance (attention) layer on 8 TRN2 NeuronCores.

Sharding: data-parallel over batch. B=8 -> one batch element per core.
Each core computes a full multi-head self-attention for its [L=1024, D=256]
slice with H=8 heads of dim 64. No collectives needed.

Per-core algorithm (fp16 matmul operands; ScalarE exp is the pacing engine):
  xT    = transpose(x) on the PE (vs identity; also warms the HAM clock
          gate early), drains split between ScalarE and DVE
  qkT   = xT + peT               (pos-enc, host-precomputed, DVE)
  qT    = Wq.T @ qkT + bq        per head-pair tiles [128(d), 1024(l)]
  kT2   = Wk.T @ qkT             per head-pair tiles [128(d), 1024(m)]
  v     = xT.T @ Wv              [m, 8, 64+1] tiles, ones column for Z
  S^T   = per-head kT2/qT matmuls, K=64, issued as CONCURRENT row-tiled
          pairs (heads 2j/2j+1 on PE row-groups 0-1/2-3 via base_partition
          0/64) -> 2x S throughput vs a zero-padded K=128 formulation.
  exp   = ScalarE, FD=1536: S chunks fill two ping-pong PSUM "generation"
          tiles of [128, 1536] (3 banks each; PSUM dep tracking is
          whole-tile, so each generation is its own pool tile) and each
          full generation is exp'd in one ACT call; ScalarE does nothing
          else mid-stream. K/Q projections for j>=1 ride every other
          generation as a [proj, S, S] mix (exp reads the S suffix) so the
          exp stream never bubbles. K0/Q0/V projections pipeline through
          the O-PSUM banks, which are idle until the first O quad.
  O     = e.T @ [v_h | 1] accumulated over 8 m-chunks into [128, 260] PSUM
          (2 banks double-buffered), emitted as 4-MM chain-slices
          interleaved two per S-pair (a long O burst in front of the
          in-order PE queue would stall the exp stream and let the PE go
          HAM-cold). Chains are strictly bank-sequential because a
          matmul's start=True clears has_written for the whole bank.
          Normalized by DVE reciprocal + broadcast multiply, DMA'd out per
          (head, quad).
Bias handling: bq added during the Q PSUM drain (DVE, per-partition
scalar); bk only shifts each score row by a constant (softmax-invariant)
so it is dropped; bv shifts the output by exactly repeat(bv, 64) because
softmax rows sum to 1, added on the host.
"""

import numpy as np
import ml_dtypes

import concourse.bass as bass
import concourse.mybir as mybir
import concourse.tile as tile
from concourse import bacc
from concourse.bass_utils import run_bass_kernel_spmd
from concourse.masks import make_identity

B, L, D = 8, 1024, 256
H, HD = 8, 64
J = H * HD  # 512
TEMPERATURE = 10000.0

f32 = mybir.dt.float32
bf16 = mybir.dt.float16  # fp16: same PE rate as bf16, 8x the mantissa

_CACHE = {}
LAST_RESULT = None  # BassKernelResults of the most recent run (for profiling)
TRACE = False

EXP_CHUNKS = 3    # FD=1536 per full ScalarE exp call


def _emit(tc, aps):
    nc = tc.nc
    Exp = mybir.ActivationFunctionType.Exp
    Copy = mybir.ActivationFunctionType.Copy
    x, wq, wk, wv, bqc, pet, out = (
        aps["x"], aps["wq"], aps["wk"], aps["wv"], aps["bqc"], aps["pet"], aps["out"],
    )

    xr = x.rearrange("(n p) c -> p n c", p=128)          # [128, 8, 256]
    petr = pet.rearrange("(t p) l -> t p l", p=128)      # [2, 128, 1024]
    wqr = wq.rearrange("(t p) j -> t p j", p=128)        # [2, 128, 512]
    wkr = wk.rearrange("(t p) j -> t p j", p=128)
    wvr = wv.rearrange("(t p) j -> t p j", p=128)
    outr = out.rearrange("(n p) j -> p n j", p=128)      # [128, 8, 512]

    import contextlib
    ctx = contextlib.ExitStack()
    persist = ctx.enter_context(tc.tile_pool(name="persist", bufs=1))
    epool = ctx.enter_context(tc.tile_pool(name="epool", bufs=10))
    rpool = ctx.enter_context(tc.tile_pool(name="rpool", bufs=4))
    s_ps = ctx.enter_context(tc.tile_pool(name="sps", bufs=2, space="PSUM"))
    o_ps = ctx.enter_context(tc.tile_pool(name="ops", bufs=2, space="PSUM"))

    # --- ACT exp-table preload (off the attention critical path) ---
    sc_in = persist.tile([128, 8], f32, name="sc_in")
    sc_out = persist.tile([128, 8], f32, name="sc_out")
    nc.vector.memset(sc_in[:], 0.0)
    nc.scalar.activation(sc_out[:], sc_in[:], Exp)

    # --- input DMAs: x on the sync HWDGE queue, pe + wk on the scalar
    # HWDGE queue (ScalarE is idle until the first exp), wq behind x on
    # sync, wv/bq on gpsimd behind the identity build (each gpsimd DMA
    # issue costs ~700ns of engine time, so the identity must come first
    # or the transposes start 5us late) ---
    ident = persist.tile([128, 128], bf16, name="ident")
    make_identity(nc, ident)

    x_sb = persist.tile([128, 8, 256], bf16, name="x_sb")
    for qtr in range(4):
        nc.sync.dma_start(out=x_sb[:, qtr * 2:(qtr + 1) * 2, :],
                          in_=xr[:, qtr * 2:(qtr + 1) * 2, :])
    pe_sb = [persist.tile([128, 1024], bf16, name=f"pe_sb{t}") for t in range(2)]
    for t in range(2):
        nc.scalar.dma_start(out=pe_sb[t][:], in_=petr[t])
    w_sb = {}
    for wname in ("wq", "wk", "wv"):
        w_sb[wname] = [
            persist.tile([128, 512], bf16, name=f"{wname}_sb{t}") for t in range(2)
        ]
    for t in range(2):
        nc.scalar.dma_start(out=w_sb["wk"][t][:], in_=wkr[t])
    for t in range(2):
        nc.sync.dma_start(out=w_sb["wq"][t][:], in_=wqr[t])
    for t in range(2):
        nc.gpsimd.dma_start(out=w_sb["wv"][t][:], in_=wvr[t])
    bq_sb = persist.tile([128, 4], f32, name="bq_sb")
    nc.gpsimd.dma_start(out=bq_sb[:], in_=bqc[:, :])

    # --- transpose x via PE; one generation tile per xT half, drains
    # split between ScalarE and DVE ---
    xT = [persist.tile([128, 1024], bf16, name=f"xT{t}") for t in range(2)]
    for c2 in range(2):
        tp = s_ps.tile([128, EXP_CHUNKS * 512], f32, tag="s", name="tp")
        for n in range(8):
            nc.tensor.matmul(
                tp[:, n * 128:(n + 1) * 128],
                lhsT=x_sb[:, n, c2 * 128:(c2 + 1) * 128],
                rhs=ident[:],
                start=True,
                stop=True,
            )
        nc.scalar.activation(xT[c2][:, 0:512], tp[:, 0:512], Copy)
        nc.vector.tensor_copy(xT[c2][:, 512:1024], tp[:, 512:1024])

    # qkT adds split per (t, l-half) so each fires as soon as its
    # transpose-drain half lands
    qkT = [persist.tile([128, 1024], bf16, name=f"qkT{t}") for t in range(2)]
    for g in range(2):
        for t in range(2):
            sl = slice(g * 512, (g + 1) * 512)
            nc.vector.tensor_add(qkT[t][:, sl], xT[t][:, sl], pe_sb[t][:, sl])

    # --- persistent SBUF operands ---
    kT2 = [persist.tile([128, 1024], bf16, name=f"kT2{j}") for j in range(4)]
    qT = [persist.tile([128, 1024], bf16, name=f"qT{j}") for j in range(4)]
    v_sb = [persist.tile([128, 8, 65], bf16, name=f"v_sb{m}") for m in range(8)]
    out_sb = persist.tile([128, 8, 512], f32, name="out_sb")

    # --- projection chunk bodies (target PSUM AP supplied by the host
    # slot: an o_ps tile early on, or a mixed generation's first chunk) ---
    def k_mm(j, l2, c):
        for c2 in range(2):
            nc.tensor.matmul(
                c,
                lhsT=w_sb["wk"][c2][:, j * 128:(j + 1) * 128],
                rhs=qkT[c2][:, l2 * 512:(l2 + 1) * 512],
                start=(c2 == 0),
                stop=(c2 == 1),
            )

    def k_drain(j, l2, c, eng):
        if eng is nc.scalar:
            nc.scalar.activation(kT2[j][:, l2 * 512:(l2 + 1) * 512], c, Copy)
        else:
            nc.vector.tensor_copy(kT2[j][:, l2 * 512:(l2 + 1) * 512], c)

    def q_mm(j, l2, c):
        for c2 in range(2):
            nc.tensor.matmul(
                c,
                lhsT=w_sb["wq"][c2][:, j * 128:(j + 1) * 128],
                rhs=qkT[c2][:, l2 * 512:(l2 + 1) * 512],
                start=(c2 == 0),
                stop=(c2 == 1),
            )

    def q_drain(j, l2, c, eng):
        nc.vector.tensor_scalar_add(
            qT[j][:, l2 * 512:(l2 + 1) * 512], c, bq_sb[:, j:j + 1]
        )

    def v_body(m, c):
        for c2 in range(2):
            nc.tensor.matmul(
                c,
                lhsT=xT[c2][:, m * 128:(m + 1) * 128],
                rhs=w_sb["wv"][c2][:],
                start=(c2 == 0),
                stop=(c2 == 1),
            )
        nc.vector.tensor_copy(
            v_sb[m][:, :, 0:64], c.rearrange("p (h d) -> p h d", h=8)
        )
        nc.vector.memset(v_sb[m][:, :, 64:65], 1.0)

    def o_proj(body):
        c = o_ps.tile([128, 512], f32, tag="o", name="pc")
        body(c)

    # --- PSUM generations: two ping-pong tiles of [128, 1536]. PSUM dep
    # tracking is whole-tile, so each generation is its own pool tile; the
    # 2-slot rotation gives one exp of pipeline slack. Mixed generations
    # put one projection chunk FIRST (cols 0:512, drained by DVE), then
    # two S chunks; exp reads the S suffix. ---
    from collections import deque
    proj_sched = deque()   # (mm_body, drain_body) for K/Q (j >= 1)
    gen = {"tile": None, "off": 0, "cap": EXP_CHUNKS, "keys": [], "pure": 0,
           "drain": None}

    def open_gen():
        gen["tile"] = s_ps.tile([128, EXP_CHUNKS * 512], f32, tag="s", name="sg")
        gen["keys"] = []
        if proj_sched and gen["pure"] >= 1:
            mm_body, drain_body = proj_sched.popleft()
            c = gen["tile"][:, 0:512]
            mm_body(c)
            # the drain is a whole-tile READER: defer it until after the
            # gen's S matmuls + exp, or they would WAR-wait on it
            gen["drain"] = (drain_body, c)
            gen["off"] = 1
            gen["cap"] = EXP_CHUNKS - 1
            gen["pure"] = 0
        else:
            gen["drain"] = None
            gen["off"] = 0
            gen["cap"] = EXP_CHUNKS
            gen["pure"] += 1

    def gen_chunk():
        if gen["tile"] is None:
            open_gen()
        n = gen["off"] + len(gen["keys"])
        return gen["tile"][:, n * 512:(n + 1) * 512]

    epos = {}       # (h, mc, l2) -> (e_tile, col_offset)

    def flush_exp(force=False):
        nS = len(gen["keys"])
        if nS < gen["cap"] and not force:
            return
        off = gen["off"]
        e = epool.tile([128, EXP_CHUNKS * 512], bf16, tag="e", name="e")
        nc.scalar.activation(
            e[:, 0:nS * 512], gen["tile"][:, off * 512:(off + nS) * 512],
            Exp, scale=float(HD) ** -0.5,
        )
        for i, key in enumerate(gen["keys"]):
            epos[key] = (e, i * 512)
        if gen["drain"] is not None:
            drain_body, c = gen["drain"]
            drain_body(c)
        gen["tile"] = None
        gen["keys"] = []
        gen["drain"] = None

    def s_pair(j, mc, l2):
        """Concurrent row-tiled S matmuls for heads 2j (rows 0:64) and
        2j+1 (rows 64:128); each writes one generation chunk."""
        msl = slice(mc * 128, (mc + 1) * 128)
        lsl = slice(l2 * 512, (l2 + 1) * 512)
        for half in range(2):
            c = gen_chunk()
            rows = slice(64 * half, 64 * half + 64)
            nc.tensor.matmul(
                c,
                lhsT=kT2[j][rows, msl],
                rhs=qT[j][rows, lsl],
                start=True,
                stop=True,
            )
            gen["keys"].append((2 * j + half, mc, l2))
            flush_exp()

    # --- O accumulation: quad (h, q) covers l-chunks 4q..4q+3.
    # start=True clears has_written for the whole PSUM bank, so the four
    # column-group chains must run chain-sequential per bank. ---
    oq = {}  # (h, q) -> {"pO": tile}

    def o_start(h, q):
        oq[(h, q)] = {
            "pO": o_ps.tile([128, 260], f32, tag="o", name="pO"),
        }

    def o_chains(h, q, mc_hold=None):
        """Emit the quad's 32 MMs chain-by-chain. mc_hold orders each
        chain's matmul on the given mc last (it is the chunk exp'd last)."""
        st = oq[(h, q)]
        mcs = list(range(8))
        if mc_hold is not None:
            mcs = [m for m in mcs if m != mc_hold] + [mc_hold]
        for g in range(4):
            for i, mc in enumerate(mcs):
                e, off = epos[(h, mc, q)]
                nc.tensor.matmul(
                    st["pO"][:, 65 * g:65 * g + 65],
                    lhsT=e[:, off + g * 128:off + (g + 1) * 128],
                    rhs=v_sb[mc][:, h, :],
                    start=(i == 0),
                    stop=(i == 7),
                )

    def o_finish(h, q, dma_engine=None):
        st = oq.pop((h, q))
        pO = st["pO"]
        hsl = slice(h * 64, (h + 1) * 64)
        pOr = pO.rearrange("p (g c) -> p g c", g=4)      # [128, 4, 65]
        rc = rpool.tile([128, 4], f32, tag="rc", name="rc")
        nc.vector.reciprocal(rc[:], pOr[:, :, 64])
        rcb = bass.AP(
            tensor=rc.tensor, offset=rc.offset,
            ap=[rc.ap[0], rc.ap[1], [0, 64]],
        )
        nc.vector.tensor_mul(
            out_sb[:, 4 * q:4 * q + 4, hsl], pOr[:, :, 0:64], rcb
        )
        eng = dma_engine or (nc.sync if (h + q) % 2 == 0 else nc.gpsimd)
        eng.dma_start(
            out=outr[:, 4 * q:4 * q + 4, hsl],
            in_=out_sb[:, 4 * q:4 * q + 4, hsl],
        )

    # O work is emitted in 4-MM slices (half a column-group chain) so the
    # in-order PE queue never holds a long O burst in front of the S
    # matmuls that feed ScalarE.
    owork = []

    def enqueue_quads(hqs):
        slices = []
        for h, q in hqs:
            o_start(h, q)
            slices.append([(h, q, g, half) for g in range(4) for half in range(2)])
        for tup in zip(*slices):
            owork.extend(tup)

    def emit_slice():
        if not owork:
            return
        h, q, g, half = owork.pop(0)
        st = oq[(h, q)]
        for i in range(4):
            mc = 4 * half + i
            e, off = epos[(h, mc, q)]
            nc.tensor.matmul(
                st["pO"][:, 65 * g:65 * g + 65],
                lhsT=e[:, off + g * 128:off + (g + 1) * 128],
                rhs=v_sb[mc][:, h, :],
                start=(half == 0 and i == 0),
                stop=(half == 1 and i == 3),
            )
        if g == 3 and half == 1:
            o_finish(h, q)

    # ---------------- schedule ----------------
    # Head: the two gating projections (K0 l2=0, Q0 l2=0) first on the two
    # free O-PSUM banks, then their l2=1 halves and the first V chunks.
    def kq_head(j, l2, c):
        k_mm(j, l2, c)
        k_drain(j, l2, c, nc.scalar)

    def qq_head(j, l2, c):
        q_mm(j, l2, c)
        q_drain(j, l2, c, nc.vector)

    o_proj(lambda c: kq_head(0, 0, c))
    o_proj(lambda c: qq_head(0, 0, c))
    o_proj(lambda c: kq_head(0, 1, c))
    o_proj(lambda c: qq_head(0, 1, c))
    o_proj(lambda c: v_body(0, c))
    o_proj(lambda c: v_body(1, c))

    # K/Q for j >= 1 ride mixed generations, one chunk every other gen.
    for j in range(1, 4):
        proj_sched.append((lambda c, j=j: k_mm(j, 0, c),
                           lambda c, j=j: k_drain(j, 0, c, nc.vector)))
        proj_sched.append((lambda c, j=j: k_mm(j, 1, c),
                           lambda c, j=j: k_drain(j, 1, c, nc.vector)))
        proj_sched.append((lambda c, j=j: q_mm(j, 0, c),
                           lambda c, j=j: q_drain(j, 0, c, nc.vector)))
        proj_sched.append((lambda c, j=j: q_mm(j, 1, c),
                           lambda c, j=j: q_drain(j, 1, c, nc.vector)))

    # Remaining V projections pipeline through the O-PSUM banks during
    # group 0 (the first O quad is not enqueued until group 1).
    v_plan = {0: 2, 1: 3, 2: 4, 3: 5, 4: 6, 5: 7}  # pair-idx -> v chunk

    # O quads run one group behind their exps; the final pair's q=1 quads
    # are emitted after the last group with mc=7 held last.
    quads_plan = {
        1: [(0, 0), (1, 0)],
        2: [(0, 1), (1, 1)],
        3: [(2, 0), (3, 0)],
        4: [(2, 1), (3, 1)],
        5: [(4, 0), (5, 0)],
        6: [(4, 1), (5, 1)],
        7: [(6, 0), (7, 0)],
    }

    groups = [(j, l2) for j in range(4) for l2 in range(2)]
    for gi, (j, l2) in enumerate(groups):
        if gi in quads_plan:
            enqueue_quads(quads_plan[gi])
        for mc in range(8):
            s_pair(j, mc, l2)
            if gi == 0 and mc in v_plan:
                o_proj(lambda c, m=v_plan[mc]: v_body(m, c))
            emit_slice()
            emit_slice()
        while owork:
            emit_slice()

    if gen["keys"]:
        flush_exp(force=True)
    o_start(6, 1)
    o_start(7, 1)
    o_chains(6, 1, mc_hold=7)
    o_chains(7, 1, mc_hold=7)
    o_finish(6, 1, dma_engine=nc.sync)
    o_finish(7, 1, dma_engine=nc.gpsimd)

    ctx.close()


def _build():
    if "nc" in _CACHE:
        return _CACHE["nc"]
    nc = bacc.Bacc("TRN2", target_bir_lowering=False, debug=False, num_devices=8)
    aps = {
        "x": nc.dram_tensor("x", [L, D], bf16, kind="ExternalInput").ap(),
        "wq": nc.dram_tensor("wq", [D, J], bf16, kind="ExternalInput").ap(),
        "wk": nc.dram_tensor("wk", [D, J], bf16, kind="ExternalInput").ap(),
        "wv": nc.dram_tensor("wv", [D, J], bf16, kind="ExternalInput").ap(),
        "bqc": nc.dram_tensor("bqc", [128, 4], f32, kind="ExternalInput").ap(),
        "pet": nc.dram_tensor("pet", [D, L], bf16, kind="ExternalInput").ap(),
        "out": nc.dram_tensor("out", [L, J], f32, kind="ExternalOutput").ap(),
    }
    with tile.TileContext(nc) as tc:
        _emit(tc, aps)
    nc.compile()
    _CACHE["nc"] = nc
    return nc


def _pe_T():
    embed = np.arange(L, dtype=np.float32)
    dim_t = np.arange(D, dtype=np.float32)
    dim_t = (np.float32(TEMPERATURE) ** (2.0 * np.floor(dim_t / 2.0) / np.float32(D))).astype(np.float32)
    pos = embed[:, None] / dim_t  # [L, D]
    pe = np.stack([np.sin(pos[:, 0::2]), np.cos(pos[:, 1::2])], axis=2).reshape(L, D)
    return np.ascontiguousarray(pe.T.astype(np.float32))  # [D, L]


def kernel(**inputs):
    global LAST_RESULT
    bf = np.float16
    x = np.asarray(inputs["x"], dtype=np.float32).astype(bf)
    wq = np.ascontiguousarray(np.asarray(inputs["Wq"], dtype=np.float32).astype(bf))
    wk = np.ascontiguousarray(np.asarray(inputs["Wk"], dtype=np.float32).astype(bf))
    wv = np.ascontiguousarray(np.asarray(inputs["Wv"], dtype=np.float32).astype(bf))
    bq = np.asarray(inputs["bq"], dtype=np.float32)
    bv = np.asarray(inputs["bv"], dtype=np.float32)

    nc = _build()
    bqc = np.ascontiguousarray(np.repeat(bq, HD).reshape(4, 128).T)  # [128, 4]
    pet = _pe_T().astype(bf)
    base = {"wq": wq, "wk": wk, "wv": wv, "bqc": bqc, "pet": pet}
    in_maps = [{**base, "x": np.ascontiguousarray(x[b])} for b in range(B)]
    res = run_bass_kernel_spmd(
        nc, in_maps, core_ids=list(range(B)), trace=TRACE
    )
    LAST_RESULT = res
    out = np.stack([res.results[b]["out"] for b in range(B)]).astype(np.float32)
    out += np.repeat(bv, HD)[None, None, :]
    return out
